# revision 62
# baseline (speedup 1.0000x reference)
"""Trainium2 Bass kernel for the multi-agent attention critic.

Strategy: data-parallel over the batch dim across 8 NeuronCores
(4096 samples/core). Inside each core everything is computed
feature-major ([feature, batch] tiles, batch on the free dim) in bf16
with fp32 PSUM accumulation:

  - self branch:  x1 = relu(W1.T xT + b1); x2 = relu(W2.T x1 + b2);
                  x3s = w3s.T x2 (kept in PSUM)
  - per agent a:  enc_a = relu(We_a.T inps_a + be_a)
                  keys_a = Wk.T enc_a (PSUM), vals_a = relu(Wv.T enc_a + bv)
                  prod_a = sel * keys_a (DVE), logits rows via indicator
                  matmuls accumulated into a [104, Bt] PSUM tile whose rows
                  are spread so agents occupy distinct 32-partition PE
                  groups (tile_position packing -> concurrent matmuls)
  - softmax over agents (logit row of (h,a) = 32*(a%4) + 4*(a//4) + h):
    E = exp(L); segsum via indicator matmul; ls = ln(segsum);
    L -= bcast(ls) via a -1-indicator matmul; w = exp(L)
    (exp/ln/relu/copy all live in one ACT table set)
  - ov: wbc_a = bcast(w rows) via indicator matmul (PSUM),
        P_a = vals_a * wbc_a (DVE), and the sum over agents is folded
        into the w3_others matmul by accumulating into the x3 PSUM tile.
  - out = Wout.T relu(x3s + x3o) + bout

The tile loop is software-pipelined in two stages: stage A (input DMA,
self branch, encoders/keys/vals, logits) of tile t+1 is emitted before
stage B (softmax, weighted values, head) of tile t, so each engine's
in-order instruction stream interleaves tile t's serial softmax tail
with tile t+1's dense matmul front. The last two tiles' B phases are
interleaved with each other on disjoint PSUM pools to shorten the
pipeline drain. All constants ship in two packed
DMAs to keep the prologue short. The 1/sqrt(d) attention scale is
folded into Wq on the host.
"""

import numpy as np
import ml_dtypes

B = 32768
NA = 8
A = NA - 1
OBS = 64
ACTD = 14
OTH_IN = 82
H_SELF = 64
H_OTH = 128
H2 = 64
HEADS = 4
AD = H_OTH // HEADS  # 32
NCORES = 8
BC = B // NCORES     # 4096 samples per core
BT = 512             # batch tile (free dim per matmul)
NT = BC // BT        # 8 tiles per core
X_IN = OBS + ACTD    # 78
A_SPLIT = 3          # agents emitted in stage-A front vs back

BF16 = ml_dtypes.bfloat16

_CACHE = {}


def _split_sync_waits(nc):
    """This walrus build rejects instructions carrying too many sem-wait
    conditions ("Too many sync wait commands"): 2 for compute instructions,
    1 for CTRL ops (Drain etc). Split extra waits onto preceding same-engine
    NOPs — engines execute their own stream in order, so a wait on an
    earlier NOP is equivalent."""
    import concourse.mybir as mybir

    n_added = 0
    for fn in nc.m.functions:
        for bb in fn.blocks:
            out = []
            for inst in bb.instructions:
                max_waits = 1
                si = inst.sync_info
                if si is not None and si.on_wait and len(si.on_wait) > max_waits:
                    waits = list(si.on_wait)
                    si.on_wait = waits[:max_waits]
                    rest = waits[max_waits:]
                    for k in range(0, len(rest), 1):
                        nop = mybir.InstNoOp(
                            name=f"{inst.name}-ws{k}", ins=[], outs=[],
                            bass_nofuse=True)
                        nop.engine = inst.engine
                        nop.sync_info = mybir.SyncInfo(
                            on_wait=[rest[k]], on_update=[])
                        out.append(nop)
                        n_added += 1
                out.append(inst)
            bb.instructions[:] = out
    return n_added


LROWS = 104  # logits PSUM tile partition count (all used rows < 100)

# Packed-constant layouts: (name, rows, cols). Offsets 4-col aligned.
_CONSTS_BF16 = [
    ("w1", 78, 64), ("w2", 64, 64), ("w3s", 64, 64), ("wq", 64, 128),
    ("we", 82, 7 * 128), ("wk", 128, 128), ("wv", 128, 128),
    ("w3o", 128, 64), ("wout", 64, 1), ("sind0", 128, LROWS),
    ("sindp", 128, 48), ("t4", LROWS, 4), ("wbclo", 128, 128),
    ("wbchi", 128, 128),
]
_CONSTS_F32 = [
    ("b1", 64, 1), ("b2", 64, 1), ("be", 128, 7), ("bv", 128, 1),
    ("bout", 1, 1), ("negind", 4, LROWS),
]


def _pack_layout(spec):
    off, w = {}, 0
    for name, rows, cols in spec:
        off[name] = w
        w += (cols + 3) // 4 * 4
    return off, w


def _const_view(spec, off, name):
    for n, rows, cols in spec:
        if n == name:
            return rows, off[name], off[name] + cols
    raise KeyError(name)


def _lbase(a):
    """Partition base of agent a's 4 logits rows (spread layout: agents sit
    in distinct 32-row PE groups so segred/wbc matmuls can run concurrently
    via tile_position packing)."""
    return 32 * (a % 4) + 4 * (a // 4)


def _indicator_constants():
    """Constant indicator matrices for the attention bookkeeping.
    Logits row of (h, a) = _lbase(a) + h."""
    # sind0[hd, m]: agent-0 segred lhsT — writes agent 0's 4 rows AND zeros
    # every other row of the logits tile (start=True full overwrite).
    sind0 = np.zeros((H_OTH, LROWS), dtype=BF16)
    for hd in range(H_OTH):
        sind0[hd, _lbase(0) + hd // AD] = 1.0
    # sindp[hd, a-1, c]: segred lhsT for agents 1..6. Each writes an
    # M=8 slice at the 32-aligned base 32*(a%4); its own 4 rows get head
    # indicators, the other 4 rows (another agent's) get zeros (+0 under
    # PSUM accumulation).
    sindp = np.zeros((H_OTH, A - 1, 8), dtype=BF16)
    for hd in range(H_OTH):
        for a in range(1, A):
            off = 4 * (a // 4)  # 0 for agents 1-3, 4 for agents 4-6
            sindp[hd, a - 1, off + hd // AD] = 1.0
    # t4[p, h]: segsum lhsT — sums logits rows of head h over agents.
    t4 = np.zeros((LROWS, HEADS), dtype=BF16)
    for a in range(A):
        for h in range(HEADS):
            t4[_lbase(a) + h, h] = 1.0
    # negind[h, p]: subtract-broadcast lhsT (fp32) — L[p] -= ls[h(p)].
    negind = np.zeros((HEADS, LROWS), dtype=np.float32)
    for a in range(A):
        for h in range(HEADS):
            negind[h, _lbase(a) + h] = -1.0
    # Broadcast lhsTs, sliced per agent at 32-aligned bases:
    #  wbc_lo: agents 0-3 slice rows [32a, 32a+4) = head indicators (K=4).
    #  wbc_hi: agents 4-6 slice rows [32j, 32j+8), j=a-4: first 4 rows zero
    #          (they are agent j's w rows), last 4 = head indicators (K=8).
    wbc_lo = np.zeros((128, H_OTH), dtype=BF16)
    wbc_hi = np.zeros((128, H_OTH), dtype=BF16)
    for a in range(4):
        for h in range(HEADS):
            for hd in range(H_OTH):
                if hd // AD == h:
                    wbc_lo[32 * a + h, hd] = 1.0
    for a in range(4, A):
        j = a - 4
        for h in range(HEADS):
            for hd in range(H_OTH):
                if hd // AD == h:
                    wbc_hi[32 * j + 4 + h, hd] = 1.0
    return sind0, sindp, t4, negind, wbc_lo, wbc_hi


def _build_nc(reps=1):
    import concourse.bass as bass
    import concourse.mybir as mybir
    import concourse.tile as tile
    from contextlib import ExitStack

    dt = mybir.dt
    AF = mybir.ActivationFunctionType

    nc = bass.Bass("TRN2", target_bir_lowering=False, debug=False)

    # ---- DRAM I/O ------------------------------------------------------
    xt = nc.dram_tensor("xt", [X_IN, BC], dt.bfloat16, kind="ExternalInput")
    ot = nc.dram_tensor("ot", [OTH_IN, A, BC], dt.bfloat16, kind="ExternalInput")
    # All constants packed into two [128, W] arrays so the prologue is two
    # DMAs instead of twenty (each small DMA costs ~1-2us of queue time
    # before compute can start).
    cb_off, cb_w = _pack_layout(_CONSTS_BF16)
    cf_off, cf_w = _pack_layout(_CONSTS_F32)
    cb = nc.dram_tensor("cb", [128, cb_w], dt.bfloat16, kind="ExternalInput")
    cf = nc.dram_tensor("cf", [128, cf_w], dt.float32, kind="ExternalInput")

    out_d = nc.dram_tensor("out", [1, BC], dt.float32, kind="ExternalOutput")

    with tile.TileContext(nc) as tc, ExitStack() as ctx:
        singles = ctx.enter_context(tc.tile_pool(name="singles", bufs=1))

        s_cb = singles.tile([128, cb_w], dt.bfloat16, name="s_cb")
        nc.sync.dma_start(out=s_cb, in_=cb.ap())
        s_cf = singles.tile([128, cf_w], dt.float32, name="s_cf")
        nc.sync.dma_start(out=s_cf, in_=cf.ap())

        def bslice(name, rows=None):
            r, c0, c1 = _const_view(_CONSTS_BF16, cb_off, name)
            return s_cb[: (rows or r), c0:c1]

        def fslice(name, rows=None):
            r, c0, c1 = _const_view(_CONSTS_F32, cf_off, name)
            return s_cf[: (rows or r), c0:c1]

        s_w1 = bslice("w1")
        s_w2 = bslice("w2")
        s_w3s = bslice("w3s")
        s_wq = bslice("wq")
        s_wk = bslice("wk")
        s_wv = bslice("wv")
        s_w3o = bslice("w3o")
        s_wout = bslice("wout")
        s_sind0 = bslice("sind0")
        s_t4 = bslice("t4")
        s_b1 = fslice("b1")
        s_b2 = fslice("b2")
        s_be = fslice("be")
        s_bv = fslice("bv")
        s_bout = fslice("bout")
        s_negind = fslice("negind")
        # multi-part constants addressed by column arithmetic
        _, we0, _ = _const_view(_CONSTS_BF16, cb_off, "we")
        _, sp0, _ = _const_view(_CONSTS_BF16, cb_off, "sindp")
        _, lo0, _ = _const_view(_CONSTS_BF16, cb_off, "wbclo")
        _, hi0, _ = _const_view(_CONSTS_BF16, cb_off, "wbchi")

        def s_we(a):
            return s_cb[:OTH_IN, we0 + a * H_OTH: we0 + (a + 1) * H_OTH]

        def s_sindp(a):
            return s_cb[:, sp0 + (a - 1) * 8: sp0 + a * 8]

        def s_wbclo(base):
            return s_cb[base:base + 4, lo0:lo0 + H_OTH]

        def s_wbchi(base):
            return s_cb[base:base + 8, hi0:hi0 + H_OTH]

        # SBUF working pools
        p_in = ctx.enter_context(tc.tile_pool(name="p_in", bufs=3))
        p_ot = ctx.enter_context(tc.tile_pool(name="p_ot", bufs=3))
        p_act = ctx.enter_context(tc.tile_pool(name="p_act", bufs=3))
        p_enc = ctx.enter_context(tc.tile_pool(name="p_enc", bufs=3))
        p_prod = ctx.enter_context(tc.tile_pool(name="p_prod", bufs=3))
        p_vals = ctx.enter_context(tc.tile_pool(name="p_vals", bufs=3))
        p_pa = ctx.enter_context(tc.tile_pool(name="p_pa", bufs=3))
        p_sm = ctx.enter_context(tc.tile_pool(name="p_sm", bufs=3))

        outs_all = singles.tile([1, BC], dt.float32, name="outs_all")

        # PSUM pools (8 banks total)
        ps1 = ctx.enter_context(tc.tile_pool(name="ps1", bufs=2, space="PSUM"))
        pskv = ctx.enter_context(tc.tile_pool(name="pskv", bufs=2, space="PSUM"))
        pswb = ctx.enter_context(tc.tile_pool(name="pswb", bufs=1, space="PSUM"))
        psx3 = ctx.enter_context(tc.tile_pool(name="psx3", bufs=1, space="PSUM"))
        psl = ctx.enter_context(tc.tile_pool(name="psl", bufs=2, space="PSUM"))

        NTOT = NT * reps

        def stage_a(t):
            """Inputs, self branch, per-agent encoders/keys/vals, logits."""
            b0 = (t % NT) * BT
            xts = p_in.tile([X_IN, BT], dt.bfloat16, tag="xts")
            nc.sync.dma_start(out=xts, in_=xt.ap()[:, b0:b0 + BT])
            ots = p_ot.tile([OTH_IN, A, BT], dt.bfloat16, tag="ots")
            nc.sync.dma_start(out=ots, in_=ot.ap()[:, :, b0:b0 + BT])

            x1p = ps1.tile([H_SELF, BT], dt.float32, tag="m", name="x1p")
            nc.tensor.matmul(x1p, s_w1, xts, start=True, stop=True)
            x1 = p_act.tile([H_SELF, BT], dt.bfloat16, tag="x1")
            nc.scalar.activation(x1, x1p, AF.Relu, bias=s_b1)

            x2p = ps1.tile([H_SELF, BT], dt.float32, tag="m", name="x2p")
            nc.tensor.matmul(x2p, s_w2, x1, start=True, stop=True)
            x2 = p_act.tile([H_SELF, BT], dt.bfloat16, tag="x2")
            nc.scalar.activation(x2, x2p, AF.Relu, bias=s_b2)

            selp = ps1.tile([H_OTH, BT], dt.float32, tag="m", name="selp")
            nc.tensor.matmul(selp, s_wq, x1, start=True, stop=True)
            sel = p_act.tile([H_OTH, BT], dt.bfloat16, tag="sel")
            nc.scalar.activation(sel, selp, AF.Copy)

            lp = psl.tile([LROWS, BT], dt.float32, tag="l", name="lp")
            vals = p_vals.tile([H_OTH, A, BT], dt.bfloat16, tag="vals")

            st = {"x2": x2, "lp": lp, "vals": vals, "b0": b0,
                  "sel": sel, "ots": ots}
            for a in range(A_SPLIT):
                agent_step(st, a)
            return st

        def agent_step(st, a):
            ots, sel, vals, lp = st["ots"], st["sel"], st["vals"], st["lp"]
            encp = ps1.tile([H_OTH, BT], dt.float32, tag="m", name="encp")
            nc.tensor.matmul(encp, s_we(a), ots[:, a, :],
                             start=True, stop=True)
            enc = p_enc.tile([H_OTH, BT], dt.bfloat16, tag="enc")
            nc.scalar.activation(enc, encp, AF.Relu, bias=s_be[:, a:a + 1])

            keysp = pskv.tile([H_OTH, BT], dt.float32, tag="kv",
                              name="keysp")
            nc.tensor.matmul(keysp, s_wk, enc, start=True, stop=True)
            valsp = pskv.tile([H_OTH, BT], dt.float32, tag="kv",
                              name="valsp")
            nc.tensor.matmul(valsp, s_wv, enc, start=True, stop=True)
            if a >= 4:
                # rebalance: ACT is the busiest engine; DVE dual-op
                # tensor_scalar does bias-add + relu in one pass.
                nc.vector.tensor_scalar(
                    out=vals[:, a, :], in0=valsp, scalar1=s_bv,
                    scalar2=0.0, op0=mybir.AluOpType.add,
                    op1=mybir.AluOpType.max)
            else:
                nc.scalar.activation(vals[:, a, :], valsp, AF.Relu,
                                     bias=s_bv)

            prod = p_prod.tile([H_OTH, BT], dt.bfloat16, tag="prod")
            nc.vector.tensor_mul(out=prod, in0=sel, in1=keysp)
            # segred: agent 0 writes all LROWS rows (zeros elsewhere,
            # start=True); agents 1..6 accumulate 4-row slices,
            # col-packed into distinct 32-partition groups so the PE
            # runs them concurrently.
            if a == 0:
                nc.tensor.matmul(lp, s_sind0, prod,
                                 start=True, stop=False,
                                 skip_group_check=True)
            else:
                base = 32 * (a % 4)
                nc.tensor.matmul(lp[base:base + 8, :],
                                 s_sindp(a), prod,
                                 start=False, stop=(a == A - 1),
                                 tile_position=(0, base),
                                 skip_group_check=True)

        def stage_a_back(st):
            for a in range(A_SPLIT, A):
                agent_step(st, a)

        def softmax_part(st, ssp_pool, ssp_tag):
            """E = exp(L), segsum, ls = ln, L -= bcast(ls), w = exp(L)."""
            lp = st["lp"]
            ee = p_sm.tile([LROWS, BT], dt.bfloat16, tag="ee")
            nc.scalar.activation(ee, lp, AF.Exp)
            ssp = ssp_pool.tile([HEADS, BT], dt.float32, tag=ssp_tag,
                                name="ssp")
            nc.tensor.matmul(ssp, s_t4, ee, start=True, stop=True)
            ls = p_sm.tile([HEADS, BT], dt.float32, tag="ls")
            nc.scalar.activation(ls, ssp, AF.Ln)
            nc.tensor.matmul(lp, s_negind, ls, start=False, stop=True,
                             skip_group_check=True)
            w = p_sm.tile([LROWS, BT], dt.bfloat16, tag="w")
            nc.scalar.activation(w, lp, AF.Exp)
            return w

        def ov_step(st, w, x3p, a, wb_pool, wb_tag):
            """One agent's broadcast matmul, product, and w3o accumulate."""
            base = 32 * (a % 4)
            wbcp = wb_pool.tile([H_OTH, BT], dt.float32, tag=wb_tag,
                                name="wbcp")
            if a < 4:
                nc.tensor.matmul(wbcp, s_wbclo(base), w[base:base + 4, :],
                                 start=True, stop=True,
                                 tile_position=(base, 0))
            else:
                nc.tensor.matmul(wbcp, s_wbchi(base), w[base:base + 8, :],
                                 start=True, stop=True,
                                 tile_position=(base, 0))
            pa = p_pa.tile([H_OTH, BT], dt.bfloat16, tag="pa")
            nc.vector.tensor_mul(out=pa, in0=st["vals"][:, a, :], in1=wbcp)
            nc.tensor.matmul(x3p, s_w3o, pa, start=False, stop=(a == A - 1),
                             skip_group_check=True)

        def head_part(st, x3p, outp_pool, outp_tag):
            x3 = p_act.tile([H2, BT], dt.bfloat16, tag="x3s")
            nc.scalar.activation(x3, x3p, AF.Relu)
            outp = outp_pool.tile([1, BT], dt.float32, tag=outp_tag,
                                  name="outp")
            nc.tensor.matmul(outp, s_wout, x3, start=True, stop=True)
            nc.scalar.activation(outs_all[:, st["b0"]:st["b0"] + BT], outp,
                                 AF.Identity, bias=s_bout)

        def stage_b_softmax(st):
            st["w"] = softmax_part(st, psx3, "x3")

        def stage_b_ov(st):
            x3p = psx3.tile([H2, BT], dt.float32, tag="x3")
            nc.tensor.matmul(x3p, s_w3s, st["x2"], start=True, stop=False,
                             skip_group_check=True)
            for a in range(A):
                ov_step(st, st["w"], x3p, a, pswb, "wb")
            head_part(st, x3p, pswb, "wb")

        def stage_b_tail_pair(stA, stB):
            """Last two tiles: no stage_a follows, so interleave their B
            phases against each other. stB uses the now-idle stage-a PSUM
            pools (ps1 ring + kv) so the two chains share no banks."""
            wA = softmax_part(stA, psx3, "x3")
            wB = softmax_part(stB, ps1, "m")
            x3pA = psx3.tile([H2, BT], dt.float32, tag="x3", name="x3pA")
            nc.tensor.matmul(x3pA, s_w3s, stA["x2"], start=True, stop=False,
                             skip_group_check=True)
            x3pB = ps1.tile([H2, BT], dt.float32, tag="m", name="x3pB")
            nc.tensor.matmul(x3pB, s_w3s, stB["x2"], start=True, stop=False,
                             skip_group_check=True)
            for a in range(A):
                ov_step(stA, wA, x3pA, a, pswb, "wb")
                ov_step(stB, wB, x3pB, a, pskv, "kv")
            head_part(stA, x3pA, pswb, "wb")
            head_part(stB, x3pB, ps1, "m")

        # 2-stage software pipeline with fine-grained interleave: tile t's
        # stage-B softmax and ov segments are emitted BETWEEN the two halves
        # of tile t+1's stage A, so every engine's (in-order) stream
        # alternates serial-chain segments with dense matmul material. The
        # final two B phases are interleaved with each other instead.
        prev = None
        last_two = None
        for t in range(NTOT):
            stf = stage_a(t)
            mid = prev if (prev is not None and t < NTOT - 1) else None
            if mid is not None:
                stage_b_softmax(mid)
            stage_a_back(stf)
            if mid is not None:
                stage_b_ov(mid)
                prev = stf
            elif prev is None:
                prev = stf
            else:
                last_two = (prev, stf)
        stage_b_tail_pair(*last_two)

        nc.sync.dma_start(out=out_d.ap(), in_=outs_all)

    _split_sync_waits(nc)
    return nc


def _prep_inputs(state_one, act_one, state_others, act_others,
                 W1, b1, W2, b2, w3_self, We, be,
                 Wk, Wq, Wv, bv, w3_others, Wout, bout):
    """Host-side sharding + layout transforms. Returns per-core in_maps."""
    scale = 1.0 / np.sqrt(np.float32(AD))

    xt_full = np.ascontiguousarray(
        np.concatenate([state_one, act_one], axis=1).T
    ).astype(BF16)                                     # [78, B]
    inps = np.concatenate([state_others, act_others], axis=2)  # [A, B, 82]
    ot_full = np.ascontiguousarray(np.transpose(inps, (2, 0, 1))).astype(BF16)

    def headcat(wm):  # [H, J, AD] -> [J, H*AD]
        return np.ascontiguousarray(
            np.transpose(np.asarray(wm, np.float32), (1, 0, 2))
            .reshape(wm.shape[1], HEADS * AD))

    sind0, sindp, t4, negind, wbc_lo, wbc_hi = _indicator_constants()

    vals_bf16 = {
        "w1": np.asarray(W1, np.float32).astype(BF16),
        "w2": np.asarray(W2, np.float32).astype(BF16),
        "w3s": np.asarray(w3_self, np.float32).astype(BF16),
        "wq": (headcat(Wq) * scale).astype(BF16),
        "we": np.ascontiguousarray(
            np.transpose(np.asarray(We, np.float32), (1, 0, 2))
            .reshape(OTH_IN, A * H_OTH)).astype(BF16),
        "wk": headcat(Wk).astype(BF16),
        "wv": headcat(Wv).astype(BF16),
        "w3o": np.asarray(w3_others, np.float32).astype(BF16),
        "wout": np.asarray(Wout, np.float32).astype(BF16),
        "sind0": sind0,
        "sindp": sindp.reshape(H_OTH, (A - 1) * 8),
        "t4": t4,
        "wbclo": wbc_lo,
        "wbchi": wbc_hi,
    }
    vals_f32 = {
        "b1": np.asarray(b1, np.float32).reshape(H_SELF, 1),
        "b2": np.asarray(b2, np.float32).reshape(H_SELF, 1),
        "be": np.ascontiguousarray(np.asarray(be, np.float32).T),
        "bv": np.asarray(bv, np.float32).reshape(HEADS * AD, 1),
        "bout": np.asarray(bout, np.float32).reshape(1, 1),
        "negind": negind,
    }

    def pack(spec, values, dtype):
        off, width = _pack_layout(spec)
        arr = np.zeros((128, width), dtype=dtype)
        for name, rows, cols in spec:
            v = values[name]
            assert v.shape == (rows, cols), (name, v.shape, rows, cols)
            arr[:rows, off[name]:off[name] + cols] = v
        return arr

    cb = pack(_CONSTS_BF16, vals_bf16, BF16)
    cf = pack(_CONSTS_F32, vals_f32, np.float32)

    in_maps = []
    for c in range(NCORES):
        sl = slice(c * BC, (c + 1) * BC)
        m = {"cb": cb, "cf": cf,
             "xt": np.ascontiguousarray(xt_full[:, sl]),
             "ot": np.ascontiguousarray(ot_full[:, :, sl])}
        in_maps.append(m)
    return in_maps


def get_nc(reps=1):
    key = ("nc", reps)
    if key not in _CACHE:
        _CACHE[key] = _build_nc(reps)
    return _CACHE[key]


def kernel(**inputs) -> np.ndarray:
    from concourse.bass_utils import run_bass_kernel_spmd

    nc = get_nc()
    in_maps = _prep_inputs(**inputs)
    res = run_bass_kernel_spmd(nc, in_maps, core_ids=list(range(NCORES)))
    out = np.concatenate(
        [np.asarray(res.results[c]["out"], np.float32).reshape(BC, 1)
         for c in range(NCORES)], axis=0)
    return out
